# revision 1
# baseline (speedup 1.0000x reference)
"""CropAndResize (tf.image.crop_and_resize semantics, bilinear, extrap=0)
Trainium2 Bass kernel, data-parallel over 8 NeuronCores.

Full inputs:  img (4,512,64,64) f32, rois (4,300,4) f32, input_image (4,3,1024,1024) f32
Full output:  (4,300,512,7,7) f32

Sharding: core c handles image n = c//2 and that image's roi slice
[ (c%2)*150 : (c%2)*150+150 ] (padded to 160 = 10 batches of 16).

Per-core device program (fp16 compute, f32 in/out):
  1. img NCHW f32 -> SBUF -> cast fp16 -> xbar DMA-transpose into the
     gather layout img_g[p, j, c] = row(hw=j*128+p), 1024B/row.
  2. RoI prep on DVE in [49pt, 160roi] layout: sample coords, clipped
     floors, lerp weights folded with the validity mask, gather indices.
  3. Index wrap into the dma_gather int16 [16-wrapped, replicated] layout
     and weight flatten to a per-corner row, via small DMAs.
  4. Per batch (16 rois): 4 SBUF-source transpose-mode dma_gathers
     (channels land on partitions), PE ones-matmul broadcast of weights,
     DVE lerp: out = sum_k T_k * W_k, write f32 out with strided DMA.
"""

import os
import sys

import numpy as np

_RL_REPO_CANDIDATES = ["/opt/trn_rl_repo", "/root/.axon_site/_ro/trn_rl_repo"]
for _p in _RL_REPO_CANDIDATES:
    if os.path.isdir(_p) and _p not in sys.path:
        sys.path.insert(0, _p)

import ml_dtypes  # noqa: E402

# ---------------------------------------------------------------- constants
N_CORES = 8
N, C, H, W = 4, 512, 64, 64
B = 300
POOL = 7
PTS = POOL * POOL  # 49
IH, IW = 1024.0, 1024.0
R_CORE = B // 2          # 150 real rois per core
R_PAD = 160              # padded roi count per core
RB = 8                   # rois per batch
NBATCH = R_PAD // RB     # 10
SLOT = 64                # per-roi slot stride in the gather index space
NIDX = RB * SLOT         # 1024 gather indices per batch (mult of 128)
SPB = NIDX // 16         # 64 wrapped-idx free slots per batch
HW = H * W               # 4096

_prog_cache = {}


def _build_program():
    import concourse.bass as bass
    import concourse.bacc as bacc
    import concourse.mybir as mybir
    import concourse.tile as tile

    f32 = mybir.dt.float32
    f16 = mybir.dt.float16
    i16 = mybir.dt.int16
    Alu = mybir.AluOpType

    nc = bacc.Bacc("TRN2", target_bir_lowering=False, debug=False,
                   num_devices=N_CORES)

    img_in = nc.dram_tensor("img", (C, HW), f32, kind="ExternalInput")
    rois_in = nc.dram_tensor("rois", (R_PAD, 4), f32, kind="ExternalInput")
    consts_in = nc.dram_tensor("consts", (1, 256), f32, kind="ExternalInput")
    out_t = nc.dram_tensor("out", (R_CORE, C, POOL, POOL), f32,
                           kind="ExternalOutput")

    with tile.TileContext(nc) as tc:
        _body(tc, nc, bass, mybir, tile, img_in, rois_in, consts_in, out_t,
              f32, f16, i16, Alu)

    nc.compile()
    return nc


def _body(tc, nc, bass, mybir, tile, img_in, rois_in, consts_in, out_t,
          f32, f16, i16, Alu):
    from contextlib import ExitStack
    ctx = ExitStack()
    with ctx:
        import os as _os
        GB = int(_os.environ.get("K_GBUFS", "3"))
        WB = int(_os.environ.get("K_WBUFS", "3"))
        OB = int(_os.environ.get("K_OBUFS", "2"))
        PB = int(_os.environ.get("K_PBUFS", "4"))
        const_pool = ctx.enter_context(tc.tile_pool(name="const", bufs=1))
        prep_pool = ctx.enter_context(tc.tile_pool(name="prep", bufs=1))
        imgstage = ctx.enter_context(tc.tile_pool(name="imgstage", bufs=2))
        gather_pool = ctx.enter_context(tc.tile_pool(name="gather", bufs=GB))
        w_pool = ctx.enter_context(tc.tile_pool(name="wts", bufs=WB))
        wrow_pool = ctx.enter_context(tc.tile_pool(name="wrow", bufs=WB))
        o_pool = ctx.enter_context(tc.tile_pool(name="outs", bufs=OB))
        dram_pool = ctx.enter_context(
            tc.tile_pool(name="dram", bufs=1, space="DRAM"))
        psum_pool = ctx.enter_context(
            tc.tile_pool(name="psum", bufs=PB, space="PSUM"))

        # ------------------------------------------------ constants
        # consts row: [0:49]=g_y per pt, [49:98]=g_x per pt, [98:226]=ones
        gy_col = const_pool.tile([PTS, 1], f32, tag="gy")
        gx_col = const_pool.tile([PTS, 1], f32, tag="gx")
        # strided loads: partition p <- consts[0, p] / consts[0, 49+p]
        nc.sync.dma_start(gy_col[:, :], consts_in.ap()[0:1, 0:PTS].rearrange(
            "a p -> p a"))
        nc.sync.dma_start(gx_col[:, :], consts_in.ap()[0:1, PTS:2 * PTS]
                          .rearrange("a p -> p a"))
        ones_f32 = const_pool.tile([1, 128], f32, tag="ones32")
        nc.sync.dma_start(ones_f32[:, :], consts_in.ap()[0:1, 98:226])
        ones16 = const_pool.tile([1, 128], f16, tag="ones16")
        nc.vector.tensor_copy(ones16[:, :], ones_f32[:, :])

        # ------------------------------------------------ image prep
        # img_nhwc[hw, c] fp16 in DRAM; built via SBUF xbar transpose:
        # img_g[p, j, c] = img row hw=j*128+p -> DRAM row-major write
        img_nhwc = dram_pool.tile([HW, C], f16, name="img_nhwc")
        img_g = const_pool.tile([128, 32, C], f16, tag="imgg")
        for cs in range(4):
            stage32 = imgstage.tile([128, HW], f32, tag="s32")
            nc.sync.dma_start(stage32[:, :],
                              img_in.ap()[cs * 128:(cs + 1) * 128, :])
            stage16 = imgstage.tile([128, HW], f16, tag="s16")
            nc.vector.tensor_copy(stage16[:, :], stage32[:, :])
            nc.sync.dma_start_transpose(
                img_g[:, :, cs * 128:(cs + 1) * 128], stage16[:, :])
        nc.sync.dma_start(
            img_nhwc[:, :].rearrange("(j p) c -> p j c", p=128),
            img_g[:, :, :])

        # ------------------------------------------------ roi prep
        # roisT[0, c*R_PAD + r] = rois[r, c]
        roisT = prep_pool.tile([1, 4 * R_PAD], f32, tag="roisT")
        nc.sync.dma_start(roisT[:, :].rearrange("o (c r) -> o c r", c=4),
                          rois_in.ap()[:, :].rearrange("r c -> c r"))

        # bc row: [y1n, x1n, dyn, dxn] each R_PAD wide
        bc = prep_pool.tile([64, 4 * R_PAD], f32, tag="bc")
        inv_h = 1.0 / (IH - 1.0)
        inv_w = 1.0 / (IW - 1.0)
        r0 = bc[0:1, :]
        nc.vector.tensor_scalar_mul(r0[:, 0:R_PAD], roisT[:, 0:R_PAD], inv_h)
        nc.vector.tensor_scalar_mul(r0[:, R_PAD:2 * R_PAD],
                                    roisT[:, R_PAD:2 * R_PAD], inv_w)
        tmp = prep_pool.tile([1, 2 * R_PAD], f32, tag="tmp2")
        nc.vector.tensor_scalar_mul(tmp[:, 0:R_PAD],
                                    roisT[:, 2 * R_PAD:3 * R_PAD], inv_h)
        nc.vector.tensor_scalar_mul(tmp[:, R_PAD:2 * R_PAD],
                                    roisT[:, 3 * R_PAD:4 * R_PAD], inv_w)
        nc.vector.tensor_sub(r0[:, 2 * R_PAD:3 * R_PAD], tmp[:, 0:R_PAD],
                             r0[:, 0:R_PAD])
        nc.vector.tensor_sub(r0[:, 3 * R_PAD:4 * R_PAD],
                             tmp[:, R_PAD:2 * R_PAD], r0[:, R_PAD:2 * R_PAD])
        # broadcast row 0 -> 64 partitions (need 49) by doubling
        for k in range(6):
            lo, hi = 1 << k, min(2 << k, 64)
            nc.sync.dma_start(bc[lo:hi, :], bc[0:lo, :][0:hi - lo, :])

        y1n = bc[0:PTS, 0:R_PAD]
        x1n = bc[0:PTS, R_PAD:2 * R_PAD]
        dyn = bc[0:PTS, 2 * R_PAD:3 * R_PAD]
        dxn = bc[0:PTS, 3 * R_PAD:4 * R_PAD]

        def ptile(nm, dt_=None):
            return prep_pool.tile([PTS, R_PAD], dt_ or f32, tag=nm, name=nm)

        def prep_axis(ax, gcol, lo_n, d_n, hdim):
            """returns (c0f, cbf, lc, mc) tiles [49, R_PAD] f32"""
            inn = ptile(f"inn{ax}")
            nc.vector.scalar_tensor_tensor(inn[:, :], d_n, gcol[:, :], lo_n,
                                           Alu.mult, Alu.add)
            nc.vector.tensor_scalar_mul(inn[:, :], inn[:, :], hdim - 1.0)
            cc = ptile(f"cc{ax}")
            nc.vector.tensor_scalar(cc[:, :], inn[:, :], 0.0, hdim - 1.0,
                                    Alu.max, Alu.min)
            # exact floor for 0<=x<2^22: t=(x+2^23)-2^23 is round-nearest;
            # subtract 1 where t > x
            rnd = ptile(f"rnd{ax}")
            nc.vector.tensor_scalar(rnd[:, :], cc[:, :], 8388608.0, 8388608.0,
                                    Alu.add, Alu.subtract)
            gt = ptile(f"gt{ax}")
            nc.vector.tensor_tensor(gt[:, :], rnd[:, :], cc[:, :], Alu.is_gt)
            c0f = ptile(f"c0f{ax}")
            nc.vector.tensor_sub(c0f[:, :], rnd[:, :], gt[:, :])
            cbf = ptile(f"cbf{ax}")
            nc.vector.tensor_scalar(cbf[:, :], c0f[:, :], 1.0, hdim - 1.0,
                                    Alu.add, Alu.min)
            lc = ptile(f"lc{ax}")
            nc.vector.tensor_sub(lc[:, :], inn[:, :], c0f[:, :])
            m1 = ptile(f"m1{ax}")
            nc.vector.tensor_scalar(m1[:, :], inn[:, :], 0.0, None, Alu.is_ge)
            m2 = ptile(f"m2{ax}")
            nc.vector.tensor_scalar(m2[:, :], inn[:, :], hdim - 1.0, None,
                                    Alu.is_le)
            mc = ptile(f"mc{ax}")
            nc.vector.tensor_mul(mc[:, :], m1[:, :], m2[:, :])
            return inn, c0f, cbf, lc, mc

        _, y0f, ybf, ly, my = prep_axis("y", gy_col, y1n, dyn, float(H))
        _, x0f, xbf, lx, mx = prep_axis("x", gx_col, x1n, dxn, float(W))

        def ab(ax_, lc, mc):
            a = ptile(f"a{ax_}")
            nc.vector.tensor_scalar(a[:, :], lc[:, :], -1.0, 1.0, Alu.mult,
                                    Alu.add)
            nc.vector.tensor_mul(a[:, :], a[:, :], mc[:, :])
            b = ptile(f"b{ax_}")
            nc.vector.tensor_mul(b[:, :], lc[:, :], mc[:, :])
            return a, b

        ay, by = ab("y", ly, my)
        ax, bx = ab("x", lx, mx)

        # per-corner weights (fp16) and indices (int16)
        corners = []  # (w16 tile, idx16 tile)
        for kc, (wy, wx_, yf, xf) in enumerate(
                ((ay, ax, y0f, x0f), (ay, bx, y0f, xbf),
                 (by, ax, ybf, x0f), (by, bx, ybf, xbf))):
            w16 = ptile(f"w16_{kc}", f16)
            nc.vector.tensor_mul(w16[:, :], wy[:, :], wx_[:, :])
            idxf = ptile(f"idxf{kc}")
            nc.vector.scalar_tensor_tensor(idxf[:, :], yf[:, :], float(W),
                                           xf[:, :], Alu.mult, Alu.add)
            idx16 = ptile(f"idx16_{kc}", i16)
            nc.vector.tensor_copy(idx16[:, :], idxf[:, :])
            corners.append((w16, idx16))

        # ------------------------------------------------ idx wrap + W flatten
        # gather order within batch b: j = rl*64 + pt  (rl<16, pt<49 valid)
        # wrapped: partition p = pt%16 (q=pt//16<4), slot s = rl*4 + q
        # idxw[k] free layout: [b(10), s(64)]
        idxw = const_pool.tile([128, 4, NBATCH, SPB], i16, tag="idxw")
        nc.gpsimd.memset(idxw[:, :, :, :], 0)
        # wflat: partition k holds corner k's flat row [b(10), rl(16), pt-slot(64)]
        wdram = dram_pool.tile([4, NBATCH * NIDX], f16, name="wdram")
        for k, (w16, idx16) in enumerate(corners):
            # idx wrap: dst[p, k, b, rl*4+q] = idx16[q*16+p, b*16+rl]
            for q in range(4):
                npq = min(16, PTS - q * 16)  # 16,16,16,1
                src = idx16[q * 16:q * 16 + npq, :].rearrange(
                    "p (b r) -> p b r", b=NBATCH)
                dst = idxw[0:npq, k, :, :].rearrange(
                    "p b (r q) -> p b r q", q=4)[:, :, :, q]
                nc.sync.dma_start(dst, src)
            # w flatten: wflat[k, b*1024 + rl*64 + pt] = w16[pt, b*16+rl]
            # dst iterated (s, b, r) to match src element order (p, b, r)
            dstw = wdram[k:k + 1, :].rearrange(
                "o (b r s) -> o s b r", b=NBATCH, r=RB)[:, 0:PTS, :, :]
            nc.sync.dma_start(dstw, w16[:, :].rearrange(
                "p (b r) -> p b r", b=NBATCH))
        for k in range(3):
            lo, hi = 16 << k, 32 << k
            nc.sync.dma_start(idxw[lo:hi, :, :, :], idxw[0:hi - lo, :, :, :])

        # ------------------------------------------------ main loop
        for b in range(NBATCH):
            # rois beyond R_CORE are host-side padding; skip fully-pad batches
            nv = RB if (b + 1) * RB <= R_CORE else R_CORE - b * RB
            if nv <= 0:
                continue
            ob = o_pool.tile([128, 4, RB, PTS], f16, tag="O")
            for k in range(4):
                tk = gather_pool.tile([128, 4, NIDX], f16, tag="T")
                nc.gpsimd.dma_gather(
                    tk[:, :, :], img_nhwc[:, :], idxw[:, k, b, :],
                    NIDX, NIDX, C,
                    transpose=True,
                )
                wrow = wrow_pool.tile([1, NIDX], f16, tag="wr")
                nc.sync.dma_start(wrow[:, :],
                                  wdram[k:k + 1, b * NIDX:(b + 1) * NIDX])
                wk = w_pool.tile([128, NIDX], f16, tag="W")
                ps = psum_pool.tile([128, NIDX], f32, tag="ps")
                nc.tensor.matmul(ps[:, :], ones16[:, :], wrow[:, :],
                                 start=True, stop=True)
                nc.scalar.copy(wk[:, :], ps[:, :])
                # valid-slot views [128, 4, RB, PTS]
                tv = tk[:, :, :].rearrange("p e (r s) -> p e r s",
                                           r=RB)[:, :, :, 0:PTS]
                wv = wk[:, :].rearrange("p (r s) -> p r s",
                                        r=RB)[:, :, 0:PTS]
                wv4 = wv  # broadcast over e by explicit per-e ops
                if k == 0:
                    for e in range(4):
                        nc.vector.tensor_mul(ob[:, e, :, :], tv[:, e, :, :],
                                             wv4)
                else:
                    for e in range(4):
                        nc.vector.tensor_mul(tv[:, e, :, :], tv[:, e, :, :],
                                             wv4)
                    nc.vector.tensor_add(ob[:, :, :, :], ob[:, :, :, :], tv)

            # output write with cast fp16 -> f32
            # dst out[b*16+rl, e*128+p, py, px]; 3-dim AP limit -> per-e DMA
            dste = out_t.ap()[b * RB:b * RB + nv, :, :, :].rearrange(
                "r (e p) py px -> p e r (py px)", e=4)
            for e in range(4):
                nc.gpsimd.dma_start(dste[:, e, :, :], ob[:, e, 0:nv, :])


def _get_program():
    if "nc" not in _prog_cache:
        _prog_cache["nc"] = _build_program()
    return _prog_cache["nc"]


def _make_consts():
    consts = np.zeros((1, 256), dtype=np.float32)
    g = (np.arange(POOL, dtype=np.float32) / np.float32(POOL - 1.0)).astype(
        np.float32)
    gy = np.repeat(g, POOL)   # g[pt//7]
    gx = np.tile(g, POOL)     # g[pt%7]
    consts[0, 0:PTS] = gy
    consts[0, PTS:2 * PTS] = gx
    consts[0, 98:226] = 1.0
    return consts


def kernel(img: np.ndarray, rois: np.ndarray,
           input_image: np.ndarray) -> np.ndarray:
    from concourse.bass_utils import run_bass_kernel_spmd

    nc = _get_program()
    consts = _make_consts()
    in_maps = []
    for c in range(N_CORES):
        n, half = c // 2, c % 2
        rpad = np.zeros((R_PAD, 4), dtype=np.float32)
        rpad[:R_CORE] = rois[n, half * R_CORE:(half + 1) * R_CORE]
        in_maps.append({
            "img": np.ascontiguousarray(
                img[n].reshape(C, HW).astype(np.float32)),
            "rois": rpad,
            "consts": consts,
        })
    res = run_bass_kernel_spmd(nc, in_maps, core_ids=list(range(N_CORES)))
    out = np.empty((N, B, C, POOL, POOL), dtype=np.float32)
    for c in range(N_CORES):
        n, half = c // 2, c % 2
        out[n, half * R_CORE:(half + 1) * R_CORE] = res.results[c]["out"]
    return out



# revision 10
# speedup vs baseline: 2.8821x; 2.8821x over previous
"""CropAndResize (tf.image.crop_and_resize semantics, bilinear, extrap=0)
Trainium2 Bass kernel, data-parallel over 8 NeuronCores.

Full inputs:  img (4,512,64,64) f32, rois (4,300,4) f32, input_image (4,3,1024,1024) f32
Full output:  (4,300,512,7,7) f32

Sharding: core c handles image n = c//2 and roi slice
[(c%2)*150 : (c%2)*150+150].

Host prep (per core, cheap O(KB) numpy on the 4-number-per-roi boxes):
  - img is transposed to row-major [hw, c] fp16 with a channel permutation
    pi(c) = (c//4) + 128*(c%4), so the transpose-mode dma_gather lands
    channel 4p+j on partition p, slot j. That makes the final output DMA
    descriptor (j,py,px) = 784B contiguous (full DMA bandwidth, no <512B
    penalty).
  - bilinear corner indices (wrapped int16 [16,*] layout, replicated to 128
    partitions) and fp16 corner weights (dense j = r*49+pt rows for the PE
    ones-broadcast) are computed from the rois in f32, matching the
    reference arithmetic step for step.

Device program per 25-roi batch (6 batches, 150 rois):
  1. one 4-corner dma_gather (num_idxs=5120, 1KiB rows) from DRAM img.
  2. per corner: PE ones-matmul broadcasts the weight row to 128
     partitions (PSUM), Act copies PSUM -> fp16 SBUF.
  3. DVE blend: ob[j] = sum_k T_k[j] * w_k  (fp16, 2x DVE mode).
  4. Act casts fp16 -> f32 into the (r, j, s) output layout.
  5. one HWDGE DMA writes out[r, 4p+j, py, px] (784B descriptors).
"""

import os
import sys

import numpy as np

_RL_REPO_CANDIDATES = ["/opt/trn_rl_repo", "/root/.axon_site/_ro/trn_rl_repo"]
for _p in _RL_REPO_CANDIDATES:
    if os.path.isdir(_p) and _p not in sys.path:
        sys.path.insert(0, _p)

# ---------------------------------------------------------------- constants
N_CORES = 8
N, C, H, W = 4, 512, 64, 64
B = 300
POOL = 7
PTS = POOL * POOL      # 49
IH, IW = 1024.0, 1024.0
HW = H * W             # 4096
R_CORE = B // 2        # 150 rois per core
RB = 15                # rois per device batch
NB = R_CORE // RB      # 10 batches
VALC = RB * PTS        # 735 valid gather rows per corner per batch
NIDXC = 768            # per-corner padded idx count (mult of 128, HW cap <1024)
NIDX = 4 * NIDXC       # 3072 gather rows per batch (all 4 corners)
SPB = NIDX // 16       # 192 wrapped idx columns per batch

_prog_cache = {}


def _build_program():
    import concourse.bass as bass
    import concourse.bacc as bacc
    import concourse.mybir as mybir
    import concourse.tile as tile

    f32 = mybir.dt.float32
    f16 = mybir.dt.float16
    i16 = mybir.dt.int16

    nc = bacc.Bacc("TRN2", target_bir_lowering=False, debug=False,
                   num_devices=N_CORES)

    img_in = nc.dram_tensor("img", (HW, C), f16, kind="ExternalInput")
    idx_in = nc.dram_tensor("idx", (128, NB * SPB), i16, kind="ExternalInput")
    wr_in = nc.dram_tensor("wrows", (1, NB * 4 * NIDXC), f16,
                           kind="ExternalInput")
    ones_in = nc.dram_tensor("ones", (1, 128), f16, kind="ExternalInput")
    out_t = nc.dram_tensor("out", (R_CORE, C, POOL, POOL), f32,
                           kind="ExternalOutput")

    with tile.TileContext(nc) as tc:
        _body(tc, nc, img_in, idx_in, wr_in, ones_in, out_t, f32, f16, i16)

    nc.compile()
    return nc


def _body(tc, nc, img_in, idx_in, wr_in, ones_in, out_t, f32, f16, i16):
    from contextlib import ExitStack
    ctx = ExitStack()
    with ctx:
        const_pool = ctx.enter_context(tc.tile_pool(name="const", bufs=1))
        g_pool = ctx.enter_context(tc.tile_pool(name="gather", bufs=2))
        w_pool = ctx.enter_context(tc.tile_pool(name="wts", bufs=3))
        wr_pool = ctx.enter_context(tc.tile_pool(name="wrow", bufs=2))
        o_pool = ctx.enter_context(tc.tile_pool(name="outs", bufs=2))
        o32_pool = ctx.enter_context(tc.tile_pool(name="outs32", bufs=2))
        psum_pool = ctx.enter_context(
            tc.tile_pool(name="psum", bufs=2, space="PSUM"))

        ones16 = const_pool.tile([1, 128], f16, tag="ones")
        nc.sync.dma_start(ones16[:, :], ones_in.ap()[:, :])
        idxs = const_pool.tile([128, NB * SPB], i16, tag="idx")
        nc.sync.dma_start(idxs[:, :], idx_in.ap()[:, :])

        SPC = NIDXC // 16  # 80 wrapped idx columns per corner
        for b in range(NB):
            tk = g_pool.tile([128, 4, 4, NIDXC], f16, tag="T")
            for k in range(4):
                nc.gpsimd.dma_gather(
                    tk[:, k, :, :], img_in.ap()[:, :],
                    idxs[:, b * SPB + k * SPC:b * SPB + (k + 1) * SPC],
                    NIDXC, NIDXC, C,
                    transpose=True,
                )
            wrow = wr_pool.tile([1, 4 * NIDXC], f16, tag="wr")
            nc.sync.dma_start(
                wrow[:, :],
                wr_in.ap()[:, b * 4 * NIDXC:(b + 1) * 4 * NIDXC])
            ob = o_pool.tile([128, 4, VALC], f16, tag="ob")
            for k in range(4):
                ps = psum_pool.tile([128, NIDXC], f32, tag="ps")
                for m0 in range(0, NIDXC, 512):
                    m1 = min(m0 + 512, NIDXC)
                    nc.tensor.matmul(
                        ps[:, m0:m1], ones16[:, :],
                        wrow[:, k * NIDXC + m0:k * NIDXC + m1],
                        start=True, stop=True)
                wk = w_pool.tile([128, NIDXC], f16, tag="W")
                nc.scalar.copy(wk[:, :], ps[:, :])
                for j in range(4):
                    tkj = tk[:, k, j, 0:VALC]
                    if k == 0:
                        nc.vector.tensor_mul(ob[:, j, :], tkj, wk[:, 0:VALC])
                    else:
                        nc.vector.tensor_mul(tkj, tkj, wk[:, 0:VALC])
                if k > 0:
                    nc.vector.tensor_add(ob[:, :, :], ob[:, :, :],
                                         tk[:, k, :, 0:VALC])

            # fp16 -> f32 cast into (r, j, s) layout on Act
            ob32 = o32_pool.tile([128, RB, 4 * PTS], f32, tag="o32")
            nc.scalar.copy(
                ob32[:, :, :].rearrange("p r (j s) -> p r j s", j=4),
                ob[:, :, :].rearrange("p j (r s) -> p r j s", r=RB))
            dste = out_t.ap()[b * RB:(b + 1) * RB, :, :, :].rearrange(
                "r (p j) py px -> p r (j py px)", j=4)
            nc.sync.dma_start(dste, ob32[:, :, :])


def _get_program():
    if "nc" not in _prog_cache:
        _prog_cache["nc"] = _build_program()
    return _prog_cache["nc"]


def _prep_image(img_n):
    """img_n (512, 64, 64) f32 -> [hw, pi(c)] fp16 row-major."""
    t = np.arange(C)
    perm = 4 * (t % 128) + t // 128      # position t holds channel perm[t]
    rows = img_n.reshape(C, HW).T        # [hw, c]
    return np.ascontiguousarray(rows[:, perm].astype(np.float16))


def _prep_rois(rois_half):
    """rois_half (150, 4) f32 -> (idxw [128, NB*SPB] i16,
    wrows [4, NB*NIDXC] f16). All arithmetic in f32 to match reference."""
    f = np.float32
    bx = rois_half.astype(f)
    y1 = bx[:, 0] / f(IH - 1.0)
    x1 = bx[:, 1] / f(IW - 1.0)
    y2 = bx[:, 2] / f(IH - 1.0)
    x2 = bx[:, 3] / f(IW - 1.0)
    g = (np.arange(POOL, dtype=f) / f(POOL - 1.0)).astype(f)
    in_y = ((y1[:, None] + (y2 - y1)[:, None] * g) * f(H - 1.0)).astype(f)
    in_x = ((x1[:, None] + (x2 - x1)[:, None] * g) * f(W - 1.0)).astype(f)

    def axis(inn, hi):
        val = ((inn >= 0.0) & (inn <= hi)).astype(f)
        c0f = np.floor(inn)
        c0 = np.clip(c0f, 0, hi).astype(np.int32)
        cb = np.minimum(c0 + 1, int(hi))
        lc = (inn - c0f).astype(f)
        wa = ((f(1.0) - lc) * val).astype(f)
        wb = (lc * val).astype(f)
        return c0, cb, wa, wb

    y0, yb, wya, wyb = axis(in_y, H - 1.0)
    x0, xb, wxa, wxb = axis(in_x, W - 1.0)

    idxw = np.zeros((128, NB * SPB), np.int16)
    wrows = np.zeros((NB, 4 * NIDXC), np.float16)
    corners = ((y0, x0, wya, wxa), (y0, xb, wya, wxb),
               (yb, x0, wyb, wxa), (yb, xb, wyb, wxb))
    idx_flat = np.zeros((NB, NIDX), np.int32)
    for k, (yc, xc, wy, wx) in enumerate(corners):
        # [150, 7py, 7px] -> per-roi flattened pt rows
        idx_full = (yc[:, :, None] * W + xc[:, None, :]).reshape(R_CORE, PTS)
        w_full = (wy[:, :, None] * wx[:, None, :]).astype(f).reshape(
            R_CORE, PTS)
        for b in range(NB):
            blk = slice(b * RB, (b + 1) * RB)
            idx_flat[b, k * NIDXC:k * NIDXC + VALC] = \
                idx_full[blk].reshape(-1)
            wrows[b, k * NIDXC:k * NIDXC + VALC] = \
                w_full[blk].reshape(-1).astype(np.float16)
    for b in range(NB):
        # wrap each corner's 1280-idx block separately: [16, 4*(NIDXC//16)]
        wr = idx_flat[b].reshape(4, NIDXC // 16, 16).transpose(2, 0, 1)
        wrapped = wr.reshape(16, SPB).astype(np.int16)
        idxw[:, b * SPB:(b + 1) * SPB] = np.tile(wrapped, (8, 1))
    return idxw, wrows


def _make_in_maps(img, rois):
    ones = np.ones((1, 128), np.float16)
    img_pm = {}
    in_maps = []
    for c in range(N_CORES):
        n, half = c // 2, c % 2
        if n not in img_pm:
            img_pm[n] = _prep_image(img[n])
        idxw, wrows = _prep_rois(
            rois[n, half * R_CORE:(half + 1) * R_CORE])
        in_maps.append({
            "img": img_pm[n],
            "idx": idxw,
            "wrows": wrows.reshape(1, -1),
            "ones": ones,
        })
    return in_maps


def kernel(img: np.ndarray, rois: np.ndarray,
           input_image: np.ndarray) -> np.ndarray:
    from concourse.bass_utils import run_bass_kernel_spmd

    nc = _get_program()
    in_maps = _make_in_maps(np.asarray(img, dtype=np.float32),
                            np.asarray(rois, dtype=np.float32))
    res = run_bass_kernel_spmd(nc, in_maps, core_ids=list(range(N_CORES)))
    out = np.empty((N, B, C, POOL, POOL), dtype=np.float32)
    for c in range(N_CORES):
        n, half = c // 2, c % 2
        out[n, half * R_CORE:(half + 1) * R_CORE] = res.results[c]["out"]
    return out


# revision 11
# speedup vs baseline: 2.8989x; 1.0058x over previous
"""CropAndResize (tf.image.crop_and_resize semantics, bilinear, extrap=0)
Trainium2 Bass kernel, data-parallel over 8 NeuronCores.

Full inputs:  img (4,512,64,64) f32, rois (4,300,4) f32, input_image (4,3,1024,1024) f32
Full output:  (4,300,512,7,7) f32

Sharding: core c handles image n = c//2 and roi slice
[(c%2)*150 : (c%2)*150+150].

Host prep (per core, cheap O(KB) numpy on the 4-number-per-roi boxes):
  - img is transposed to row-major [hw, c] fp16 with a channel permutation
    pi(c) = (c//4) + 128*(c%4), so the transpose-mode dma_gather lands
    channel 4p+j on partition p, slot j. That makes the final output DMA
    descriptor (j,py,px) = 784B contiguous (full DMA bandwidth, no <512B
    penalty).
  - bilinear corner indices (wrapped int16 [16,*] layout, replicated to 128
    partitions) and fp16 corner weights (dense j = r*49+pt rows for the PE
    ones-broadcast) are computed from the rois in f32, matching the
    reference arithmetic step for step.

Device program per 25-roi batch (6 batches, 150 rois):
  1. one 4-corner dma_gather (num_idxs=5120, 1KiB rows) from DRAM img.
  2. per corner: PE ones-matmul broadcasts the weight row to 128
     partitions (PSUM), Act copies PSUM -> fp16 SBUF.
  3. DVE blend: ob[j] = sum_k T_k[j] * w_k  (fp16, 2x DVE mode).
  4. Act casts fp16 -> f32 into the (r, j, s) output layout.
  5. one HWDGE DMA writes out[r, 4p+j, py, px] (784B descriptors).
"""

import os
import sys

import numpy as np

_RL_REPO_CANDIDATES = ["/opt/trn_rl_repo", "/root/.axon_site/_ro/trn_rl_repo"]
for _p in _RL_REPO_CANDIDATES:
    if os.path.isdir(_p) and _p not in sys.path:
        sys.path.insert(0, _p)

# ---------------------------------------------------------------- constants
N_CORES = 8
N, C, H, W = 4, 512, 64, 64
B = 300
POOL = 7
PTS = POOL * POOL      # 49
IH, IW = 1024.0, 1024.0
HW = H * W             # 4096
R_CORE = B // 2        # 150 rois per core
RB = 15                # rois per device batch
NB = R_CORE // RB      # 10 batches
VALC = RB * PTS        # 735 valid gather rows per corner per batch
NIDXC = 768            # per-corner padded idx count (mult of 128, HW cap <1024)
NIDX = 4 * NIDXC       # 3072 gather rows per batch (all 4 corners)
SPB = NIDX // 16       # 192 wrapped idx columns per batch

_prog_cache = {}


def _build_program():
    import concourse.bass as bass
    import concourse.bacc as bacc
    import concourse.mybir as mybir
    import concourse.tile as tile

    f32 = mybir.dt.float32
    f16 = mybir.dt.float16
    i16 = mybir.dt.int16

    nc = bacc.Bacc("TRN2", target_bir_lowering=False, debug=False,
                   num_devices=N_CORES)

    img_in = nc.dram_tensor("img", (HW, C), f16, kind="ExternalInput")
    idx_in = nc.dram_tensor("idx", (128, NB * SPB), i16, kind="ExternalInput")
    wr_in = nc.dram_tensor("wrows", (1, NB * 4 * NIDXC), f16,
                           kind="ExternalInput")
    ones_in = nc.dram_tensor("ones", (1, 128), f16, kind="ExternalInput")
    out_t = nc.dram_tensor("out", (R_CORE, C, POOL, POOL), f32,
                           kind="ExternalOutput")

    with tile.TileContext(nc) as tc:
        _body(tc, nc, img_in, idx_in, wr_in, ones_in, out_t, f32, f16, i16)

    nc.compile()
    return nc


def _body(tc, nc, img_in, idx_in, wr_in, ones_in, out_t, f32, f16, i16):
    from contextlib import ExitStack
    ctx = ExitStack()
    with ctx:
        const_pool = ctx.enter_context(tc.tile_pool(name="const", bufs=1))
        g_pool = ctx.enter_context(tc.tile_pool(name="gather", bufs=2))
        w_pool = ctx.enter_context(tc.tile_pool(name="wts", bufs=3))
        wr_pool = ctx.enter_context(tc.tile_pool(name="wrow", bufs=2))
        o_pool = ctx.enter_context(tc.tile_pool(name="outs", bufs=2))
        o32_pool = ctx.enter_context(tc.tile_pool(name="outs32", bufs=2))
        psum_pool = ctx.enter_context(
            tc.tile_pool(name="psum", bufs=2, space="PSUM"))

        idx_pool = ctx.enter_context(tc.tile_pool(name="idxp", bufs=NB))
        ones16 = const_pool.tile([1, 128], f16, tag="ones")
        nc.sync.dma_start(ones16[:, :], ones_in.ap()[:, :])

        SPC = NIDXC // 16  # wrapped idx columns per corner
        for b in range(NB):
            idxb = idx_pool.tile([128, SPB], i16, tag="idxb")
            nc.sync.dma_start(idxb[:, :],
                              idx_in.ap()[:, b * SPB:(b + 1) * SPB])
            tk = g_pool.tile([128, 4, 4, NIDXC], f16, tag="T")
            for k in range(4):
                nc.gpsimd.dma_gather(
                    tk[:, k, :, :], img_in.ap()[:, :],
                    idxb[:, k * SPC:(k + 1) * SPC],
                    NIDXC, NIDXC, C,
                    transpose=True,
                )
            wrow = wr_pool.tile([1, 4 * NIDXC], f16, tag="wr")
            nc.sync.dma_start(
                wrow[:, :],
                wr_in.ap()[:, b * 4 * NIDXC:(b + 1) * 4 * NIDXC])
            ob = o_pool.tile([128, 4, VALC], f16, tag="ob")
            for k in range(4):
                ps = psum_pool.tile([128, NIDXC], f32, tag="ps")
                for m0 in range(0, NIDXC, 512):
                    m1 = min(m0 + 512, NIDXC)
                    nc.tensor.matmul(
                        ps[:, m0:m1], ones16[:, :],
                        wrow[:, k * NIDXC + m0:k * NIDXC + m1],
                        start=True, stop=True)
                wk = w_pool.tile([128, NIDXC], f16, tag="W")
                nc.scalar.copy(wk[:, :], ps[:, :])
                wkb = wk[:, 0:VALC].unsqueeze(1).broadcast_to(
                    [128, 4, VALC])
                tkk = tk[:, k, :, 0:VALC]
                if k == 0:
                    nc.vector.tensor_mul(ob[:, :, :], tkk, wkb)
                else:
                    nc.vector.tensor_mul(tkk, tkk, wkb)
                    nc.vector.tensor_add(ob[:, :, :], ob[:, :, :], tkk)

            # fp16 -> f32 cast into (r, j, s) layout on Act
            ob32 = o32_pool.tile([128, RB, 4 * PTS], f32, tag="o32")
            halves = ((0, RB),) if b < NB - 1 else ((0, 8), (8, RB))
            for r0, r1 in halves:
                nc.scalar.copy(
                    ob32[:, r0:r1, :].rearrange("p r (j s) -> p r j s", j=4),
                    ob[:, :, r0 * PTS:r1 * PTS].rearrange(
                        "p j (r s) -> p r j s", r=r1 - r0))
                dste = out_t.ap()[b * RB + r0:b * RB + r1, :, :, :].rearrange(
                    "r (p j) py px -> p r (j py px)", j=4)
                nc.sync.dma_start(dste, ob32[:, r0:r1, :])


def _get_program():
    if "nc" not in _prog_cache:
        _prog_cache["nc"] = _build_program()
    return _prog_cache["nc"]


def _prep_image(img_n):
    """img_n (512, 64, 64) f32 -> [hw, pi(c)] fp16 row-major."""
    t = np.arange(C)
    perm = 4 * (t % 128) + t // 128      # position t holds channel perm[t]
    rows = img_n.reshape(C, HW).T        # [hw, c]
    return np.ascontiguousarray(rows[:, perm].astype(np.float16))


def _prep_rois(rois_half):
    """rois_half (150, 4) f32 -> (idxw [128, NB*SPB] i16,
    wrows [4, NB*NIDXC] f16). All arithmetic in f32 to match reference."""
    f = np.float32
    bx = rois_half.astype(f)
    y1 = bx[:, 0] / f(IH - 1.0)
    x1 = bx[:, 1] / f(IW - 1.0)
    y2 = bx[:, 2] / f(IH - 1.0)
    x2 = bx[:, 3] / f(IW - 1.0)
    g = (np.arange(POOL, dtype=f) / f(POOL - 1.0)).astype(f)
    in_y = ((y1[:, None] + (y2 - y1)[:, None] * g) * f(H - 1.0)).astype(f)
    in_x = ((x1[:, None] + (x2 - x1)[:, None] * g) * f(W - 1.0)).astype(f)

    def axis(inn, hi):
        val = ((inn >= 0.0) & (inn <= hi)).astype(f)
        c0f = np.floor(inn)
        c0 = np.clip(c0f, 0, hi).astype(np.int32)
        cb = np.minimum(c0 + 1, int(hi))
        lc = (inn - c0f).astype(f)
        wa = ((f(1.0) - lc) * val).astype(f)
        wb = (lc * val).astype(f)
        return c0, cb, wa, wb

    y0, yb, wya, wyb = axis(in_y, H - 1.0)
    x0, xb, wxa, wxb = axis(in_x, W - 1.0)

    idxw = np.zeros((128, NB * SPB), np.int16)
    wrows = np.zeros((NB, 4 * NIDXC), np.float16)
    corners = ((y0, x0, wya, wxa), (y0, xb, wya, wxb),
               (yb, x0, wyb, wxa), (yb, xb, wyb, wxb))
    idx_flat = np.zeros((NB, NIDX), np.int32)
    for k, (yc, xc, wy, wx) in enumerate(corners):
        # [150, 7py, 7px] -> per-roi flattened pt rows
        idx_full = (yc[:, :, None] * W + xc[:, None, :]).reshape(R_CORE, PTS)
        w_full = (wy[:, :, None] * wx[:, None, :]).astype(f).reshape(
            R_CORE, PTS)
        for b in range(NB):
            blk = slice(b * RB, (b + 1) * RB)
            idx_flat[b, k * NIDXC:k * NIDXC + VALC] = \
                idx_full[blk].reshape(-1)
            wrows[b, k * NIDXC:k * NIDXC + VALC] = \
                w_full[blk].reshape(-1).astype(np.float16)
    for b in range(NB):
        # wrap each corner's 1280-idx block separately: [16, 4*(NIDXC//16)]
        wr = idx_flat[b].reshape(4, NIDXC // 16, 16).transpose(2, 0, 1)
        wrapped = wr.reshape(16, SPB).astype(np.int16)
        idxw[:, b * SPB:(b + 1) * SPB] = np.tile(wrapped, (8, 1))
    return idxw, wrows


def _make_in_maps(img, rois):
    ones = np.ones((1, 128), np.float16)
    img_pm = {}
    in_maps = []
    for c in range(N_CORES):
        n, half = c // 2, c % 2
        if n not in img_pm:
            img_pm[n] = _prep_image(img[n])
        idxw, wrows = _prep_rois(
            rois[n, half * R_CORE:(half + 1) * R_CORE])
        in_maps.append({
            "img": img_pm[n],
            "idx": idxw,
            "wrows": wrows.reshape(1, -1),
            "ones": ones,
        })
    return in_maps


def kernel(img: np.ndarray, rois: np.ndarray,
           input_image: np.ndarray) -> np.ndarray:
    from concourse.bass_utils import run_bass_kernel_spmd

    nc = _get_program()
    in_maps = _make_in_maps(np.asarray(img, dtype=np.float32),
                            np.asarray(rois, dtype=np.float32))
    res = run_bass_kernel_spmd(nc, in_maps, core_ids=list(range(N_CORES)))
    out = np.empty((N, B, C, POOL, POOL), dtype=np.float32)
    for c in range(N_CORES):
        n, half = c // 2, c % 2
        out[n, half * R_CORE:(half + 1) * R_CORE] = res.results[c]["out"]
    return out


# revision 13
# speedup vs baseline: 2.9363x; 1.0129x over previous
"""CropAndResize (tf.image.crop_and_resize semantics, bilinear, extrap=0)
Trainium2 Bass kernel, data-parallel over 8 NeuronCores.

Full inputs:  img (4,512,64,64) f32, rois (4,300,4) f32, input_image (4,3,1024,1024) f32
Full output:  (4,300,512,7,7) f32

Sharding: core c handles image n = c//2 and roi slice
[(c%2)*150 : (c%2)*150+150].

Host prep (per core, cheap O(KB) numpy on the 4-number-per-roi boxes):
  - img is transposed to row-major [hw, c] fp16 with a channel permutation
    pi(c) = (c//4) + 128*(c%4), so the transpose-mode dma_gather lands
    channel 4p+j on partition p, slot j. That makes the final output DMA
    descriptor (j,py,px) = 784B contiguous (full DMA bandwidth, no <512B
    penalty).
  - bilinear corner indices (wrapped int16 [16,*] layout, replicated to 128
    partitions) and fp16 corner weights (dense j = r*49+pt rows for the PE
    ones-broadcast) are computed from the rois in f32, matching the
    reference arithmetic step for step.

Device program per 25-roi batch (6 batches, 150 rois):
  1. one 4-corner dma_gather (num_idxs=5120, 1KiB rows) from DRAM img.
  2. per corner: PE ones-matmul broadcasts the weight row to 128
     partitions (PSUM), Act copies PSUM -> fp16 SBUF.
  3. DVE blend: ob[j] = sum_k T_k[j] * w_k  (fp16, 2x DVE mode).
  4. Act casts fp16 -> f32 into the (r, j, s) output layout.
  5. one HWDGE DMA writes out[r, 4p+j, py, px] (784B descriptors).
"""

import os
import sys

import numpy as np

_RL_REPO_CANDIDATES = ["/opt/trn_rl_repo", "/root/.axon_site/_ro/trn_rl_repo"]
for _p in _RL_REPO_CANDIDATES:
    if os.path.isdir(_p) and _p not in sys.path:
        sys.path.insert(0, _p)

# ---------------------------------------------------------------- constants
N_CORES = 8
N, C, H, W = 4, 512, 64, 64
B = 300
POOL = 7
PTS = POOL * POOL      # 49
IH, IW = 1024.0, 1024.0
HW = H * W             # 4096
R_CORE = B // 2        # 150 rois per core
RB = 15                # rois per device batch
NB = R_CORE // RB      # 10 batches
VALC = RB * PTS        # 735 valid gather rows per corner per batch
NIDXC = 768            # per-corner padded idx count (mult of 128, HW cap <1024)
NIDX = 4 * NIDXC       # 3072 gather rows per batch (all 4 corners)
SPB = NIDX // 16       # 192 wrapped idx columns per batch

_prog_cache = {}


def _build_program():
    import concourse.bass as bass
    import concourse.bacc as bacc
    import concourse.mybir as mybir
    import concourse.tile as tile

    f32 = mybir.dt.float32
    f16 = mybir.dt.float16
    i16 = mybir.dt.int16

    nc = bacc.Bacc("TRN2", target_bir_lowering=False, debug=False,
                   num_devices=N_CORES)

    img_in = nc.dram_tensor("img", (HW, C), f16, kind="ExternalInput")
    idx_in = nc.dram_tensor("idx", (128, NB * SPB), i16, kind="ExternalInput")
    wr_in = nc.dram_tensor("wrows", (1, NB * 4 * NIDXC), f16,
                           kind="ExternalInput")
    ones_in = nc.dram_tensor("ones", (1, 128), f16, kind="ExternalInput")
    out_t = nc.dram_tensor("out", (R_CORE, C, POOL, POOL), f32,
                           kind="ExternalOutput")

    with tile.TileContext(nc) as tc:
        _body(tc, nc, img_in, idx_in, wr_in, ones_in, out_t, f32, f16, i16)

    nc.compile()
    return nc


def _body(tc, nc, img_in, idx_in, wr_in, ones_in, out_t, f32, f16, i16):
    from contextlib import ExitStack
    ctx = ExitStack()
    with ctx:
        const_pool = ctx.enter_context(tc.tile_pool(name="const", bufs=1))
        g_pool = ctx.enter_context(tc.tile_pool(name="gather", bufs=2))
        w_pool = ctx.enter_context(tc.tile_pool(name="wts", bufs=3))
        o_pool = ctx.enter_context(tc.tile_pool(name="outs", bufs=2))
        o32_pool = ctx.enter_context(tc.tile_pool(name="outs32", bufs=2))
        psum_pool = ctx.enter_context(
            tc.tile_pool(name="psum", bufs=2, space="PSUM"))

        # batch-0 idx first so the first gather's DGE starts ASAP
        idxs = const_pool.tile([128, NB * SPB], i16, tag="idx")
        nc.sync.dma_start(idxs[:, 0:SPB], idx_in.ap()[:, 0:SPB])
        nc.sync.dma_start(idxs[:, SPB:], idx_in.ap()[:, SPB:])
        wrs = const_pool.tile([1, NB * 4 * NIDXC], f16, tag="wrs")
        nc.sync.dma_start(wrs[:, :], wr_in.ap()[:, :])
        ones16 = const_pool.tile([1, 128], f16, tag="ones")
        nc.gpsimd.memset(ones16[:, :], 1.0)

        SPC = NIDXC // 16  # wrapped idx columns per corner
        for b in range(NB):
            tk = g_pool.tile([128, 4, 4, NIDXC], f16, tag="T")
            for k in range(4):
                nc.gpsimd.dma_gather(
                    tk[:, k, :, :], img_in.ap()[:, :],
                    idxs[:, b * SPB + k * SPC:b * SPB + (k + 1) * SPC],
                    NIDXC, NIDXC, C,
                    transpose=True,
                )
            ob = o_pool.tile([128, 4, VALC], f16, tag="ob")
            for k in range(4):
                ps = psum_pool.tile([128, NIDXC], f32, tag="ps")
                for m0 in range(0, NIDXC, 512):
                    m1 = min(m0 + 512, NIDXC)
                    nc.tensor.matmul(
                        ps[:, m0:m1], ones16[:, :],
                        wrs[:, (b * 4 + k) * NIDXC + m0:
                             (b * 4 + k) * NIDXC + m1],
                        start=True, stop=True)
                wk = w_pool.tile([128, NIDXC], f16, tag="W")
                nc.scalar.copy(wk[:, :], ps[:, :])
                wkb = wk[:, 0:VALC].unsqueeze(1).broadcast_to(
                    [128, 4, VALC])
                tkk = tk[:, k, :, 0:VALC]
                if k == 0:
                    nc.vector.tensor_mul(ob[:, :, :], tkk, wkb)
                else:
                    nc.vector.tensor_mul(tkk, tkk, wkb)
                    nc.vector.tensor_add(ob[:, :, :], ob[:, :, :], tkk)

            # fp16 -> f32 cast into (r, j, s) layout on Act
            ob32 = o32_pool.tile([128, RB, 4 * PTS], f32, tag="o32")
            halves = ((0, RB),) if b < NB - 1 else ((0, 8), (8, RB))
            for r0, r1 in halves:
                nc.scalar.copy(
                    ob32[:, r0:r1, :].rearrange("p r (j s) -> p r j s", j=4),
                    ob[:, :, r0 * PTS:r1 * PTS].rearrange(
                        "p j (r s) -> p r j s", r=r1 - r0))
                dste = out_t.ap()[b * RB + r0:b * RB + r1, :, :, :].rearrange(
                    "r (p j) py px -> p r (j py px)", j=4)
                nc.sync.dma_start(dste, ob32[:, r0:r1, :])


def _get_program():
    if "nc" not in _prog_cache:
        _prog_cache["nc"] = _build_program()
    return _prog_cache["nc"]


def _prep_image(img_n):
    """img_n (512, 64, 64) f32 -> [hw, pi(c)] fp16 row-major."""
    t = np.arange(C)
    perm = 4 * (t % 128) + t // 128      # position t holds channel perm[t]
    rows = img_n.reshape(C, HW).T        # [hw, c]
    return np.ascontiguousarray(rows[:, perm].astype(np.float16))


def _prep_rois(rois_half):
    """rois_half (150, 4) f32 -> (idxw [128, NB*SPB] i16,
    wrows [4, NB*NIDXC] f16). All arithmetic in f32 to match reference."""
    f = np.float32
    bx = rois_half.astype(f)
    y1 = bx[:, 0] / f(IH - 1.0)
    x1 = bx[:, 1] / f(IW - 1.0)
    y2 = bx[:, 2] / f(IH - 1.0)
    x2 = bx[:, 3] / f(IW - 1.0)
    g = (np.arange(POOL, dtype=f) / f(POOL - 1.0)).astype(f)
    in_y = ((y1[:, None] + (y2 - y1)[:, None] * g) * f(H - 1.0)).astype(f)
    in_x = ((x1[:, None] + (x2 - x1)[:, None] * g) * f(W - 1.0)).astype(f)

    def axis(inn, hi):
        val = ((inn >= 0.0) & (inn <= hi)).astype(f)
        c0f = np.floor(inn)
        c0 = np.clip(c0f, 0, hi).astype(np.int32)
        cb = np.minimum(c0 + 1, int(hi))
        lc = (inn - c0f).astype(f)
        wa = ((f(1.0) - lc) * val).astype(f)
        wb = (lc * val).astype(f)
        return c0, cb, wa, wb

    y0, yb, wya, wyb = axis(in_y, H - 1.0)
    x0, xb, wxa, wxb = axis(in_x, W - 1.0)

    idxw = np.zeros((128, NB * SPB), np.int16)
    wrows = np.zeros((NB, 4 * NIDXC), np.float16)
    corners = ((y0, x0, wya, wxa), (y0, xb, wya, wxb),
               (yb, x0, wyb, wxa), (yb, xb, wyb, wxb))
    idx_flat = np.zeros((NB, NIDX), np.int32)
    for k, (yc, xc, wy, wx) in enumerate(corners):
        # [150, 7py, 7px] -> per-roi flattened pt rows
        idx_full = (yc[:, :, None] * W + xc[:, None, :]).reshape(R_CORE, PTS)
        w_full = (wy[:, :, None] * wx[:, None, :]).astype(f).reshape(
            R_CORE, PTS)
        for b in range(NB):
            blk = slice(b * RB, (b + 1) * RB)
            idx_flat[b, k * NIDXC:k * NIDXC + VALC] = \
                idx_full[blk].reshape(-1)
            wrows[b, k * NIDXC:k * NIDXC + VALC] = \
                w_full[blk].reshape(-1).astype(np.float16)
    for b in range(NB):
        # wrap each corner's 1280-idx block separately: [16, 4*(NIDXC//16)]
        wr = idx_flat[b].reshape(4, NIDXC // 16, 16).transpose(2, 0, 1)
        wrapped = wr.reshape(16, SPB).astype(np.int16)
        idxw[:, b * SPB:(b + 1) * SPB] = np.tile(wrapped, (8, 1))
    return idxw, wrows


def _make_in_maps(img, rois):
    ones = np.ones((1, 128), np.float16)
    img_pm = {}
    in_maps = []
    for c in range(N_CORES):
        n, half = c // 2, c % 2
        if n not in img_pm:
            img_pm[n] = _prep_image(img[n])
        idxw, wrows = _prep_rois(
            rois[n, half * R_CORE:(half + 1) * R_CORE])
        in_maps.append({
            "img": img_pm[n],
            "idx": idxw,
            "wrows": wrows.reshape(1, -1),
            "ones": ones,
        })
    return in_maps


def kernel(img: np.ndarray, rois: np.ndarray,
           input_image: np.ndarray) -> np.ndarray:
    from concourse.bass_utils import run_bass_kernel_spmd

    nc = _get_program()
    in_maps = _make_in_maps(np.asarray(img, dtype=np.float32),
                            np.asarray(rois, dtype=np.float32))
    res = run_bass_kernel_spmd(nc, in_maps, core_ids=list(range(N_CORES)))
    out = np.empty((N, B, C, POOL, POOL), dtype=np.float32)
    for c in range(N_CORES):
        n, half = c // 2, c % 2
        out[n, half * R_CORE:(half + 1) * R_CORE] = res.results[c]["out"]
    return out


# revision 14
# speedup vs baseline: 3.0926x; 1.0532x over previous
"""CropAndResize (tf.image.crop_and_resize semantics, bilinear, extrap=0)
Trainium2 Bass kernel, data-parallel over 8 NeuronCores.

Full inputs:  img (4,512,64,64) f32, rois (4,300,4) f32, input_image (4,3,1024,1024) f32
Full output:  (4,300,512,7,7) f32

Sharding: core c handles image n = c//2 and roi slice
[(c%2)*150 : (c%2)*150+150].

Host prep (per core, cheap O(KB) numpy on the 4-number-per-roi boxes):
  - img is transposed to row-major [hw, c] fp16 with a channel permutation
    pi(c) = (c//4) + 128*(c%4), so the transpose-mode dma_gather lands
    channel 4p+j on partition p, slot j. That makes the final output DMA
    descriptor (j,py,px) = 784B contiguous (full DMA bandwidth, no <512B
    penalty).
  - bilinear corner indices (wrapped int16 [16,*] layout, replicated to 128
    partitions) and fp16 corner weights (dense j = r*49+pt rows for the PE
    ones-broadcast) are computed from the rois in f32, matching the
    reference arithmetic step for step.

Device program per 25-roi batch (6 batches, 150 rois):
  1. one 4-corner dma_gather (num_idxs=5120, 1KiB rows) from DRAM img.
  2. per corner: PE ones-matmul broadcasts the weight row to 128
     partitions (PSUM), Act copies PSUM -> fp16 SBUF.
  3. DVE blend: ob[j] = sum_k T_k[j] * w_k  (fp16, 2x DVE mode).
  4. Act casts fp16 -> f32 into the (r, j, s) output layout.
  5. one HWDGE DMA writes out[r, 4p+j, py, px] (784B descriptors).
"""

import os
import sys

import numpy as np

_RL_REPO_CANDIDATES = ["/opt/trn_rl_repo", "/root/.axon_site/_ro/trn_rl_repo"]
for _p in _RL_REPO_CANDIDATES:
    if os.path.isdir(_p) and _p not in sys.path:
        sys.path.insert(0, _p)

# ---------------------------------------------------------------- constants
N_CORES = 8
N, C, H, W = 4, 512, 64, 64
B = 300
POOL = 7
PTS = POOL * POOL      # 49
IH, IW = 1024.0, 1024.0
HW = H * W             # 4096
R_CORE = B // 2        # 150 rois per core
RB = 15                # rois per device batch
NB = R_CORE // RB      # 10 batches
VALC = RB * PTS        # 735 valid gather rows per corner per batch
NIDXC = 768            # per-corner padded idx count (mult of 128, HW cap <1024)
NIDX = 4 * NIDXC       # 3072 gather rows per batch (all 4 corners)
SPB = NIDX // 16       # 192 wrapped idx columns per batch

_prog_cache = {}


def _build_program():
    import concourse.bass as bass
    import concourse.bacc as bacc
    import concourse.mybir as mybir
    import concourse.tile as tile

    f32 = mybir.dt.float32
    f16 = mybir.dt.float16
    i16 = mybir.dt.int16

    nc = bacc.Bacc("TRN2", target_bir_lowering=False, debug=False,
                   num_devices=N_CORES)

    img_in = nc.dram_tensor("img", (HW, C), f16, kind="ExternalInput")
    idx_in = nc.dram_tensor("idx", (128, NB * SPB), i16, kind="ExternalInput")
    wr_in = nc.dram_tensor("wrows", (1, NB * 4 * NIDXC), f16,
                           kind="ExternalInput")
    ones_in = nc.dram_tensor("ones", (1, 128), f16, kind="ExternalInput")
    out_t = nc.dram_tensor("out", (R_CORE, C, POOL, POOL), f32,
                           kind="ExternalOutput")

    with tile.TileContext(nc) as tc:
        _body(tc, nc, img_in, idx_in, wr_in, ones_in, out_t, f32, f16, i16)

    nc.compile()
    return nc


def _body(tc, nc, img_in, idx_in, wr_in, ones_in, out_t, f32, f16, i16):
    from contextlib import ExitStack
    ctx = ExitStack()
    with ctx:
        const_pool = ctx.enter_context(tc.tile_pool(name="const", bufs=1))
        g_pool = ctx.enter_context(tc.tile_pool(name="gather", bufs=3))
        wr_pool = ctx.enter_context(tc.tile_pool(name="wrow", bufs=3))
        w_pool = ctx.enter_context(tc.tile_pool(name="wts", bufs=3))
        o_pool = ctx.enter_context(tc.tile_pool(name="outs", bufs=2))
        o32_pool = ctx.enter_context(tc.tile_pool(name="outs32", bufs=2))
        psum_pool = ctx.enter_context(
            tc.tile_pool(name="psum", bufs=2, space="PSUM"))

        # batch-0 idx first so the first gather's DGE starts ASAP
        idxs = const_pool.tile([128, NB * SPB], i16, tag="idx")
        nc.sync.dma_start(idxs[:, 0:SPB], idx_in.ap()[:, 0:SPB])
        nc.sync.dma_start(idxs[:, SPB:], idx_in.ap()[:, SPB:])
        ones16 = const_pool.tile([1, 128], f16, tag="ones")
        nc.gpsimd.memset(ones16[:, :], 1.0)

        SPC = NIDXC // 16  # wrapped idx columns per corner
        for b in range(NB):
            tk = g_pool.tile([128, 4, 4, NIDXC], f16, tag="T")
            for k in range(4):
                nc.gpsimd.dma_gather(
                    tk[:, k, :, :], img_in.ap()[:, :],
                    idxs[:, b * SPB + k * SPC:b * SPB + (k + 1) * SPC],
                    NIDXC, NIDXC, C,
                    transpose=True,
                )
            wrow = wr_pool.tile([1, 4 * NIDXC], f16, tag="wr")
            nc.sync.dma_start(
                wrow[:, :],
                wr_in.ap()[:, b * 4 * NIDXC:(b + 1) * 4 * NIDXC])
            ob = o_pool.tile([128, 4, VALC], f16, tag="ob")
            # split the last batch's blend so the tail drains sooner
            vsplits = ((0, VALC),) if b < NB - 1 else ((0, 392), (392, VALC))
            for k in range(4):
                ps = psum_pool.tile([128, NIDXC], f32, tag="ps")
                for m0 in range(0, NIDXC, 512):
                    m1 = min(m0 + 512, NIDXC)
                    nc.tensor.matmul(
                        ps[:, m0:m1], ones16[:, :],
                        wrow[:, k * NIDXC + m0:k * NIDXC + m1],
                        start=True, stop=True)
                wk = w_pool.tile([128, NIDXC], f16, tag="W")
                nc.scalar.copy(wk[:, :], ps[:, :])
                for v0, v1 in vsplits:
                    wkb = wk[:, v0:v1].unsqueeze(1).broadcast_to(
                        [128, 4, v1 - v0])
                    tkk = tk[:, k, :, v0:v1]
                    if k == 0:
                        nc.vector.tensor_mul(ob[:, :, v0:v1], tkk, wkb)
                    else:
                        nc.vector.tensor_mul(tkk, tkk, wkb)
                        nc.vector.tensor_add(ob[:, :, v0:v1],
                                             ob[:, :, v0:v1], tkk)

            # fp16 -> f32 cast into (r, j, s) layout on Act
            ob32 = o32_pool.tile([128, RB, 4 * PTS], f32, tag="o32")
            halves = ((0, RB),) if b < NB - 1 else ((0, 8), (8, RB))
            for r0, r1 in halves:
                nc.scalar.copy(
                    ob32[:, r0:r1, :].rearrange("p r (j s) -> p r j s", j=4),
                    ob[:, :, r0 * PTS:r1 * PTS].rearrange(
                        "p j (r s) -> p r j s", r=r1 - r0))
                dste = out_t.ap()[b * RB + r0:b * RB + r1, :, :, :].rearrange(
                    "r (p j) py px -> p r (j py px)", j=4)
                nc.sync.dma_start(dste, ob32[:, r0:r1, :])


def _get_program():
    if "nc" not in _prog_cache:
        _prog_cache["nc"] = _build_program()
    return _prog_cache["nc"]


def _prep_image(img_n):
    """img_n (512, 64, 64) f32 -> [hw, pi(c)] fp16 row-major."""
    t = np.arange(C)
    perm = 4 * (t % 128) + t // 128      # position t holds channel perm[t]
    rows = img_n.reshape(C, HW).T        # [hw, c]
    return np.ascontiguousarray(rows[:, perm].astype(np.float16))


def _prep_rois(rois_half):
    """rois_half (150, 4) f32 -> (idxw [128, NB*SPB] i16,
    wrows [4, NB*NIDXC] f16). All arithmetic in f32 to match reference."""
    f = np.float32
    bx = rois_half.astype(f)
    y1 = bx[:, 0] / f(IH - 1.0)
    x1 = bx[:, 1] / f(IW - 1.0)
    y2 = bx[:, 2] / f(IH - 1.0)
    x2 = bx[:, 3] / f(IW - 1.0)
    g = (np.arange(POOL, dtype=f) / f(POOL - 1.0)).astype(f)
    in_y = ((y1[:, None] + (y2 - y1)[:, None] * g) * f(H - 1.0)).astype(f)
    in_x = ((x1[:, None] + (x2 - x1)[:, None] * g) * f(W - 1.0)).astype(f)

    def axis(inn, hi):
        val = ((inn >= 0.0) & (inn <= hi)).astype(f)
        c0f = np.floor(inn)
        c0 = np.clip(c0f, 0, hi).astype(np.int32)
        cb = np.minimum(c0 + 1, int(hi))
        lc = (inn - c0f).astype(f)
        wa = ((f(1.0) - lc) * val).astype(f)
        wb = (lc * val).astype(f)
        return c0, cb, wa, wb

    y0, yb, wya, wyb = axis(in_y, H - 1.0)
    x0, xb, wxa, wxb = axis(in_x, W - 1.0)

    idxw = np.zeros((128, NB * SPB), np.int16)
    wrows = np.zeros((NB, 4 * NIDXC), np.float16)
    corners = ((y0, x0, wya, wxa), (y0, xb, wya, wxb),
               (yb, x0, wyb, wxa), (yb, xb, wyb, wxb))
    idx_flat = np.zeros((NB, NIDX), np.int32)
    for k, (yc, xc, wy, wx) in enumerate(corners):
        # [150, 7py, 7px] -> per-roi flattened pt rows
        idx_full = (yc[:, :, None] * W + xc[:, None, :]).reshape(R_CORE, PTS)
        w_full = (wy[:, :, None] * wx[:, None, :]).astype(f).reshape(
            R_CORE, PTS)
        for b in range(NB):
            blk = slice(b * RB, (b + 1) * RB)
            idx_flat[b, k * NIDXC:k * NIDXC + VALC] = \
                idx_full[blk].reshape(-1)
            wrows[b, k * NIDXC:k * NIDXC + VALC] = \
                w_full[blk].reshape(-1).astype(np.float16)
    for b in range(NB):
        # wrap each corner's 1280-idx block separately: [16, 4*(NIDXC//16)]
        wr = idx_flat[b].reshape(4, NIDXC // 16, 16).transpose(2, 0, 1)
        wrapped = wr.reshape(16, SPB).astype(np.int16)
        idxw[:, b * SPB:(b + 1) * SPB] = np.tile(wrapped, (8, 1))
    return idxw, wrows


def _make_in_maps(img, rois):
    ones = np.ones((1, 128), np.float16)
    img_pm = {}
    in_maps = []
    for c in range(N_CORES):
        n, half = c // 2, c % 2
        if n not in img_pm:
            img_pm[n] = _prep_image(img[n])
        idxw, wrows = _prep_rois(
            rois[n, half * R_CORE:(half + 1) * R_CORE])
        in_maps.append({
            "img": img_pm[n],
            "idx": idxw,
            "wrows": wrows.reshape(1, -1),
            "ones": ones,
        })
    return in_maps


def kernel(img: np.ndarray, rois: np.ndarray,
           input_image: np.ndarray) -> np.ndarray:
    from concourse.bass_utils import run_bass_kernel_spmd

    nc = _get_program()
    in_maps = _make_in_maps(np.asarray(img, dtype=np.float32),
                            np.asarray(rois, dtype=np.float32))
    res = run_bass_kernel_spmd(nc, in_maps, core_ids=list(range(N_CORES)))
    out = np.empty((N, B, C, POOL, POOL), dtype=np.float32)
    for c in range(N_CORES):
        n, half = c // 2, c % 2
        out[n, half * R_CORE:(half + 1) * R_CORE] = res.results[c]["out"]
    return out


# revision 16
# speedup vs baseline: 3.1201x; 1.0089x over previous
"""CropAndResize (tf.image.crop_and_resize semantics, bilinear, extrap=0)
Trainium2 Bass kernel, data-parallel over 8 NeuronCores.

Full inputs:  img (4,512,64,64) f32, rois (4,300,4) f32, input_image (4,3,1024,1024) f32
Full output:  (4,300,512,7,7) f32

Sharding: core c handles image n = c//2 and roi slice
[(c%2)*150 : (c%2)*150+150].

Host prep (per core, cheap O(KB) numpy on the 4-number-per-roi boxes):
  - img is transposed to row-major [hw, c] fp16 with a channel permutation
    pi(c) = (c//4) + 128*(c%4), so the transpose-mode dma_gather lands
    channel 4p+j on partition p, slot j. That makes the final output DMA
    descriptor (j,py,px) = 784B contiguous (full DMA bandwidth, no <512B
    penalty).
  - bilinear corner indices (wrapped int16 [16,*] layout, replicated to 128
    partitions) and fp16 corner weights (dense j = r*49+pt rows for the PE
    ones-broadcast) are computed from the rois in f32, matching the
    reference arithmetic step for step.

Device program per 25-roi batch (6 batches, 150 rois):
  1. one 4-corner dma_gather (num_idxs=5120, 1KiB rows) from DRAM img.
  2. per corner: PE ones-matmul broadcasts the weight row to 128
     partitions (PSUM), Act copies PSUM -> fp16 SBUF.
  3. DVE blend: ob[j] = sum_k T_k[j] * w_k  (fp16, 2x DVE mode).
  4. Act casts fp16 -> f32 into the (r, j, s) output layout.
  5. one HWDGE DMA writes out[r, 4p+j, py, px] (784B descriptors).
"""

import os
import sys

import numpy as np

_RL_REPO_CANDIDATES = ["/opt/trn_rl_repo", "/root/.axon_site/_ro/trn_rl_repo"]
for _p in _RL_REPO_CANDIDATES:
    if os.path.isdir(_p) and _p not in sys.path:
        sys.path.insert(0, _p)

# ---------------------------------------------------------------- constants
N_CORES = 8
N, C, H, W = 4, 512, 64, 64
B = 300
POOL = 7
PTS = POOL * POOL      # 49
IH, IW = 1024.0, 1024.0
HW = H * W             # 4096
R_CORE = B // 2        # 150 rois per core


def _mk_batches():
    # (roi_start, n_rois, nidxc): per-corner idx count is n*49 padded to a
    # multiple of 128; the dma_gather HW caps num_idxs below 1024, so 18
    # rois (882 -> 896) is the densest clean batch. 8x18 + 6 = 150.
    out = []
    r0 = 0
    for nr in [18] * 8 + [6]:
        nidxc = -(-(nr * PTS) // 128) * 128
        out.append((r0, nr, nidxc))
        r0 += nr
    return out


BATCHES = _mk_batches()
NB = len(BATCHES)
IDX_COLS = sum(4 * nx // 16 for _, _, nx in BATCHES)   # wrapped idx columns
WR_LEN = sum(4 * nx for _, _, nx in BATCHES)           # weight row length
RB_MAX = max(nr for _, nr, _ in BATCHES)
NXM = max(nx for _, _, nx in BATCHES)   # largest per-corner idx count

_prog_cache = {}


def _build_program():
    import concourse.bass as bass
    import concourse.bacc as bacc
    import concourse.mybir as mybir
    import concourse.tile as tile

    f32 = mybir.dt.float32
    f16 = mybir.dt.float16
    i16 = mybir.dt.int16

    nc = bacc.Bacc("TRN2", target_bir_lowering=False, debug=False,
                   num_devices=N_CORES)

    img_in = nc.dram_tensor("img", (HW, C), f16, kind="ExternalInput")
    idx_in = nc.dram_tensor("idx", (128, IDX_COLS), i16, kind="ExternalInput")
    wr_in = nc.dram_tensor("wrows", (1, WR_LEN), f16, kind="ExternalInput")
    ones_in = nc.dram_tensor("ones", (1, 128), f16, kind="ExternalInput")
    out_t = nc.dram_tensor("out", (R_CORE, C, POOL, POOL), f32,
                           kind="ExternalOutput")

    with tile.TileContext(nc) as tc:
        _body(tc, nc, img_in, idx_in, wr_in, ones_in, out_t, f32, f16, i16)

    nc.compile()
    return nc


def _body(tc, nc, img_in, idx_in, wr_in, ones_in, out_t, f32, f16, i16):
    from contextlib import ExitStack
    ctx = ExitStack()
    with ctx:
        const_pool = ctx.enter_context(tc.tile_pool(name="const", bufs=1))
        g_pool = ctx.enter_context(tc.tile_pool(name="gather", bufs=3))
        wr_pool = ctx.enter_context(tc.tile_pool(name="wrow", bufs=2))
        w_pool = ctx.enter_context(tc.tile_pool(name="wts", bufs=3))
        o_pool = ctx.enter_context(tc.tile_pool(name="outs", bufs=2))
        o32_pool = ctx.enter_context(tc.tile_pool(name="outs32", bufs=2))
        psum_pool = ctx.enter_context(
            tc.tile_pool(name="psum", bufs=2, space="PSUM"))

        # batch-0 idx first so the first gather's DGE starts ASAP
        spb0 = 4 * BATCHES[0][2] // 16
        idxs = const_pool.tile([128, IDX_COLS], i16, tag="idx")
        nc.sync.dma_start(idxs[:, 0:spb0], idx_in.ap()[:, 0:spb0])
        nc.sync.dma_start(idxs[:, spb0:], idx_in.ap()[:, spb0:])
        ones16 = const_pool.tile([1, 128], f16, tag="ones")
        nc.gpsimd.memset(ones16[:, :], 1.0)

        icol = 0
        woff = 0
        for b, (rb0, nr, nidxc) in enumerate(BATCHES):
            valc = nr * PTS
            spc = nidxc // 16
            # max-size tiles, view-carved per batch so all batches share tags
            tkf = g_pool.tile([128, 4 * 4 * NXM], f16, tag="T")
            for k in range(4):
                dst = tkf[:, k * 4 * nidxc:(k + 1) * 4 * nidxc].rearrange(
                    "p (j i) -> p j i", j=4)
                nc.gpsimd.dma_gather(
                    dst, img_in.ap()[:, :],
                    idxs[:, icol + k * spc:icol + (k + 1) * spc],
                    nidxc, nidxc, C,
                    transpose=True,
                )
            wrow = wr_pool.tile([1, 4 * NXM], f16, tag="wr")
            nc.sync.dma_start(wrow[:, 0:4 * nidxc],
                              wr_in.ap()[:, woff:woff + 4 * nidxc])
            obf = o_pool.tile([128, 4 * RB_MAX * PTS], f16, tag="ob")
            ob = obf[:, 0:4 * valc].rearrange("p (j i) -> p j i", j=4)
            for k in range(4):
                ps = psum_pool.tile([128, NXM], f32, tag="ps")
                for m0 in range(0, nidxc, 512):
                    m1 = min(m0 + 512, nidxc)
                    nc.tensor.matmul(
                        ps[:, m0:m1], ones16[:, :],
                        wrow[:, k * nidxc + m0:k * nidxc + m1],
                        start=True, stop=True)
                wk = w_pool.tile([128, NXM], f16, tag="W")
                nc.scalar.copy(wk[:, 0:nidxc], ps[:, 0:nidxc])
                wkb = wk[:, 0:valc].unsqueeze(1).broadcast_to(
                    [128, 4, valc])
                tkk = tkf[:, k * 4 * nidxc:(k + 1) * 4 * nidxc].rearrange(
                    "p (j i) -> p j i", j=4)[:, :, 0:valc]
                if k == 0:
                    nc.vector.tensor_mul(ob[:, :, :], tkk, wkb)
                else:
                    nc.vector.tensor_mul(tkk, tkk, wkb)
                    nc.vector.tensor_add(ob[:, :, :], ob[:, :, :], tkk)

            # fp16 -> f32 cast into (r, j, s) layout on Act
            o32f = o32_pool.tile([128, RB_MAX * 4 * PTS], f32, tag="o32")
            ob32 = o32f[:, 0:nr * 4 * PTS].rearrange(
                "p (r q) -> p r q", r=nr)
            halves = ((0, nr),) if nr <= 9 else ((0, nr // 2), (nr // 2, nr))
            for r0, r1 in halves:
                nc.scalar.copy(
                    ob32[:, r0:r1, :].rearrange("p r (j s) -> p r j s", j=4),
                    ob[:, :, r0 * PTS:r1 * PTS].rearrange(
                        "p j (r s) -> p r j s", r=r1 - r0))
                dste = out_t.ap()[rb0 + r0:rb0 + r1, :, :, :].rearrange(
                    "r (p j) py px -> p r (j py px)", j=4)
                nc.sync.dma_start(dste, ob32[:, r0:r1, :])
            icol += 4 * spc
            woff += 4 * nidxc


def _get_program():
    if "nc" not in _prog_cache:
        _prog_cache["nc"] = _build_program()
    return _prog_cache["nc"]


def _prep_image(img_n):
    """img_n (512, 64, 64) f32 -> [hw, pi(c)] fp16 row-major."""
    t = np.arange(C)
    perm = 4 * (t % 128) + t // 128      # position t holds channel perm[t]
    rows = img_n.reshape(C, HW).T        # [hw, c]
    return np.ascontiguousarray(rows[:, perm].astype(np.float16))


def _prep_rois(rois_half):
    """rois_half (150, 4) f32 -> (idxw [128, NB*SPB] i16,
    wrows [4, NB*NIDXC] f16). All arithmetic in f32 to match reference."""
    f = np.float32
    bx = rois_half.astype(f)
    y1 = bx[:, 0] / f(IH - 1.0)
    x1 = bx[:, 1] / f(IW - 1.0)
    y2 = bx[:, 2] / f(IH - 1.0)
    x2 = bx[:, 3] / f(IW - 1.0)
    g = (np.arange(POOL, dtype=f) / f(POOL - 1.0)).astype(f)
    in_y = ((y1[:, None] + (y2 - y1)[:, None] * g) * f(H - 1.0)).astype(f)
    in_x = ((x1[:, None] + (x2 - x1)[:, None] * g) * f(W - 1.0)).astype(f)

    def axis(inn, hi):
        val = ((inn >= 0.0) & (inn <= hi)).astype(f)
        c0f = np.floor(inn)
        c0 = np.clip(c0f, 0, hi).astype(np.int32)
        cb = np.minimum(c0 + 1, int(hi))
        lc = (inn - c0f).astype(f)
        wa = ((f(1.0) - lc) * val).astype(f)
        wb = (lc * val).astype(f)
        return c0, cb, wa, wb

    y0, yb, wya, wyb = axis(in_y, H - 1.0)
    x0, xb, wxa, wxb = axis(in_x, W - 1.0)

    idxw = np.zeros((128, IDX_COLS), np.int16)
    wrows = np.zeros((1, WR_LEN), np.float16)
    corners = ((y0, x0, wya, wxa), (y0, xb, wya, wxb),
               (yb, x0, wyb, wxa), (yb, xb, wyb, wxb))
    idx_fulls = []
    w_fulls = []
    for yc, xc, wy, wx in corners:
        idx_fulls.append(
            (yc[:, :, None] * W + xc[:, None, :]).reshape(R_CORE, PTS))
        w_fulls.append((wy[:, :, None] * wx[:, None, :]).astype(f).reshape(
            R_CORE, PTS))
    icol = 0
    woff = 0
    for rb0, nr, nidxc in BATCHES:
        valc = nr * PTS
        for k in range(4):
            flat = np.zeros(nidxc, np.int32)
            flat[:valc] = idx_fulls[k][rb0:rb0 + nr].reshape(-1)
            wrapped = flat.reshape(nidxc // 16, 16).T.astype(np.int16)
            spc = nidxc // 16
            idxw[:, icol + k * spc:icol + (k + 1) * spc] = \
                np.tile(wrapped, (8, 1))
            wrows[0, woff + k * nidxc:woff + k * nidxc + valc] = \
                w_fulls[k][rb0:rb0 + nr].reshape(-1).astype(np.float16)
        icol += 4 * spc
        woff += 4 * nidxc
    return idxw, wrows


def _make_in_maps(img, rois):
    ones = np.ones((1, 128), np.float16)
    img_pm = {}
    in_maps = []
    for c in range(N_CORES):
        n, half = c // 2, c % 2
        if n not in img_pm:
            img_pm[n] = _prep_image(img[n])
        idxw, wrows = _prep_rois(
            rois[n, half * R_CORE:(half + 1) * R_CORE])
        in_maps.append({
            "img": img_pm[n],
            "idx": idxw,
            "wrows": wrows,
            "ones": ones,
        })
    return in_maps


def kernel(img: np.ndarray, rois: np.ndarray,
           input_image: np.ndarray) -> np.ndarray:
    from concourse.bass_utils import run_bass_kernel_spmd

    nc = _get_program()
    in_maps = _make_in_maps(np.asarray(img, dtype=np.float32),
                            np.asarray(rois, dtype=np.float32))
    res = run_bass_kernel_spmd(nc, in_maps, core_ids=list(range(N_CORES)))
    out = np.empty((N, B, C, POOL, POOL), dtype=np.float32)
    for c in range(N_CORES):
        n, half = c // 2, c % 2
        out[n, half * R_CORE:(half + 1) * R_CORE] = res.results[c]["out"]
    return out
